# revision 1
# baseline (speedup 1.0000x reference)
"""DFlash draft-model kernel for 8x Trainium2 NeuronCores.

Sharding: head-parallel attention (core c owns head c) + vocab-parallel LM head
(core c owns vocab shard c), joined by a small AllGather of the normalized
per-head context. Block-sparse attention: kv tiles above each q-tile's max
anchor are skipped (anchors are sorted); boundary tiles get an anchor-compare
mask, draft-diagonal tiles get one of 4 precomputed pattern masks.

Per-core device outputs: row max + row sum(exp) of its logit shard and the
target-column logits; host combines into (loss, accuracy).
"""
import sys
sys.path.insert(0, '/opt/trn_rl_repo')
import numpy as np
import ml_dtypes

import concourse.mybir as mybir
import concourse.tile as tile
from concourse import bacc
from concourse.bass_utils import run_bass_kernel_spmd
from concourse.bass_interp import get_hw_module

F32 = mybir.dt.float32
BF16 = mybir.dt.bfloat16
BFNP = ml_dtypes.bfloat16

B, S, N, BS, D, H, V = 1, 2048, 128, 16, 512, 8, 32000
MASK_TOKEN_ID = 3
NC = 8
DH = D // H            # 64
Q = N * BS             # 2048
VS = V // NC           # 4000 vocab per core
NF = D // 128          # 4 feature chunks
QG = 4                 # q free-tiles of 512
ZC = 500               # logits psum chunk (1 psum bank)
NZC = VS // ZC         # 8 chunks per q-tile

_cache = {}
_last_in_maps = None
import os as _os
MASK_ENG = _os.environ.get("K_MASK_ENG", "vector")   # vector | gpsimd
PHASE = _os.environ.get("K_PHASE", "full")           # full | attn | lm



def _build_schedule(anc):
    sched = []
    for g in range(QG):
        blk = anc[32 * g:32 * g + 32]
        amin, amax = int(blk.min()), int(blk.max())
        lst = []
        for t in range((amax + 127) // 128):
            masked = (128 * t + 128) > amin
            lst.append((t, 1 if masked else 0, 0))
        for u in range(4):
            lst.append((16 + 4 * g + u, 2, u))
        sched.append(lst)
    return sched


def _build_program(sched, reps=1, collective=True):
    nc = bacc.Bacc("TRN2", target_bir_lowering=False, debug=False, num_devices=NC)

    din = {}
    for name, shape, dt in [
        ("i_ht", [D, Q], BF16),           # hidden^T
        ("i_estt", [128, NF * 128], BF16),  # anchor-token embeddings^T, [p, f*128+j]
        ("i_emask", [128, NF], F32),        # mask-token embedding, [p, f]
        ("i_anchorb", [128, Q], F32),       # anchor per q, bcast over partitions
        ("i_kviota", [128, 32], F32),
        ("i_dmask", [128, 4 * 512], BF16),  # 4 draft-diagonal mask tiles
        ("i_wq", [128, NF * DH], BF16),
        ("i_wk", [128, NF * DH], BF16),
        ("i_wv", [128, NF * DH], BF16),
        ("i_wo", [128, NF * D], BF16),
        ("i_wlm", [128, NF * VS], BF16),
        ("i_wt", [128, NF * Q], BF16),
    ]:
        din[name] = nc.dram_tensor(name, shape, dt, kind="ExternalInput").ap()
    o_se = nc.dram_tensor("o_se", [128, 16], F32, kind="ExternalOutput").ap()
    o_mx = nc.dram_tensor("o_mx", [128, 16], F32, kind="ExternalOutput").ap()
    o_tl = nc.dram_tensor("o_tl", [1, Q], F32, kind="ExternalOutput").ap()

    with tile.TileContext(nc) as tc:
        for _rep in range(reps):
            _emit(nc, tc, din, o_se, o_mx, o_tl, sched, collective, _rep)

    nc.compile()
    nc.m = get_hw_module(nc.m)
    return nc


def _emit(nc, tc, din, o_se, o_mx, o_tl, sched, collective, rep):
    with tc.tile_pool(name=f"persist{rep}", bufs=1) as pp, \
         tc.tile_pool(name=f"dram{rep}", bufs=1, space="DRAM") as dp:
        # ---- loads needed by projections/attention first; lm-head weights last
        anchorb = pp.tile([128, Q], F32, name="anchorb")
        nc.sync.dma_start(anchorb[:], din["i_anchorb"][:])
        kviota = pp.tile([128, 32], F32, name="kviota")
        nc.sync.dma_start(kviota[:], din["i_kviota"][:])
        estt = pp.tile([128, NF * 128], BF16, name="estt")
        nc.sync.dma_start(estt[:], din["i_estt"][:])
        emask = pp.tile([128, NF], F32, name="emask")
        nc.sync.dma_start(emask[:], din["i_emask"][:])
        wq_sb = pp.tile([128, NF * DH], BF16, name="wq_sb")
        nc.sync.dma_start(wq_sb[:], din["i_wq"][:])
        wk_sb = pp.tile([128, NF * DH], BF16, name="wk_sb")
        nc.sync.dma_start(wk_sb[:], din["i_wk"][:])
        wv_sb = pp.tile([128, NF * DH], BF16, name="wv_sb")
        nc.sync.dma_start(wv_sb[:], din["i_wv"][:])
        dmask = pp.tile([128, 4 * 512], BF16, name="dmask")
        nc.sync.dma_start(dmask[:], din["i_dmask"][:])

        # ---- X^T = [hidden^T | NE^T], 4 feature chunks [128, 4096]
        xt = []
        for f in range(NF):
            t = pp.tile([128, S + Q], BF16, name=f"xt{f}")
            nc.sync.dma_start(t[:, 0:S], din["i_ht"][128 * f:128 * (f + 1), :])
            # NE^T: fill with mask embedding, overwrite block-start columns
            nc.vector.tensor_scalar(
                t[:, S:S + Q], anchorb[:], 0.0, emask[:, f:f + 1],
                mybir.AluOpType.mult, mybir.AluOpType.add)
            dst = t[:, S:S + Q].rearrange("p (b j) -> p b j", j=BS)[:, :, 0:1]
            src = estt[:, 128 * f:128 * (f + 1)].rearrange("p (b o) -> p b o", o=1)
            nc.vector.tensor_copy(dst, src)
            xt.append(t)

        # ---- lm-head weights (big; overlap attention)
        wo_sb = pp.tile([128, NF * D], BF16, name="wo_sb")
        nc.sync.dma_start(wo_sb[:], din["i_wo"][:])
        wt_sb = pp.tile([128, NF * Q], BF16, name="wt_sb")
        nc.sync.dma_start(wt_sb[:], din["i_wt"][:])
        wlm = []
        for f in range(NF):
            t = pp.tile([128, VS], BF16, name=f"wlm{f}")
            nc.sync.dma_start(t[:], din["i_wlm"][:, VS * f:VS * (f + 1)])
            wlm.append(t)

        ones64 = pp.tile([1, DH], F32, name="ones64")
        nc.vector.memset(ones64[:], 1.0)
        onescol_f = pp.tile([128, 1], F32, name="onescol_f")
        nc.vector.memset(onescol_f[:], 1.0)

        kT = pp.tile([DH, S + Q], BF16, name="kT")
        qT = pp.tile([DH, Q], BF16, name="qT")
        vaug = pp.tile([128, 32 * (DH + 1)], BF16, name="vaug")
        nc.vector.memset(vaug[:], 1.0)
        ctxT = pp.tile([DH + 1, Q], F32, name="ctxT")
        ctxfT = [pp.tile([128, Q], BF16, name=f"ctxfT{f}") for f in range(NF)]
        outT = [pp.tile([128, Q], BF16, name=f"outT{f}") for f in range(NF)]
        se_sb = pp.tile([128, 16], F32, name="se_sb")
        mx_sb = pp.tile([128, 16], F32, name="mx_sb")
        gin = pp.tile([DH, Q], BF16, name="gin")
        recip = pp.tile([1, Q], F32, name="recip")
        tl_sb = pp.tile([1, Q], F32, name="tl_sb")
        gb_in = [dp.tile([DH, Q // 2], BF16, name=f"gb_in{h}") for h in range(2)]
        gb_out = [dp.tile([NC * DH, Q // 2], BF16, name=f"gb_out{h}",
                          addr_space="Shared" if collective else "Local")
                  for h in range(2)]

        if PHASE == "lm":
            for f in range(NF):
                nc.vector.memset(ctxfT[f][:], 0.01)
        # ---- projections (own PSUM scope, closes before attention)
        if PHASE != "lm":
         with tc.tile_pool(name=f"projps{rep}", bufs=2, space="PSUM") as projps:
            for n in range((S + Q) // 512):
                ps = projps.tile([DH, 512], F32, name="kps", tag="proj")
                for f in range(NF):
                    nc.tensor.matmul(ps[:], wk_sb[:, DH * f:DH * (f + 1)],
                                     xt[f][:, 512 * n:512 * (n + 1)],
                                     start=(f == 0), stop=(f == NF - 1))
                nc.scalar.copy(kT[:, 512 * n:512 * (n + 1)], ps[:])
            for n in range(Q // 512):
                ps = projps.tile([DH, 512], F32, name="qps", tag="proj")
                for f in range(NF):
                    nc.tensor.matmul(ps[:], wq_sb[:, DH * f:DH * (f + 1)],
                                     xt[f][:, S + 512 * n:S + 512 * (n + 1)],
                                     start=(f == 0), stop=(f == NF - 1))
                nc.scalar.copy(qT[:, 512 * n:512 * (n + 1)], ps[:])
            for T in range(32):
                ps = projps.tile([128, DH], F32, name="vps", tag="proj")
                for f in range(NF):
                    nc.tensor.matmul(ps[:], xt[f][:, 128 * T:128 * (T + 1)],
                                     wv_sb[:, DH * f:DH * (f + 1)],
                                     start=(f == 0), stop=(f == NF - 1))
                nc.scalar.copy(vaug[:, 65 * T:65 * T + DH], ps[:])

        # ---- attention + per-half normalize/AllGather, two-half pipeline
        if PHASE != "lm":
         with tc.tile_pool(name=f"scoreps{rep}", bufs=2, space="PSUM") as scoreps, \
             tc.tile_pool(name=f"ctxps{rep}", bufs=2, space="PSUM") as ctxps, \
             tc.tile_pool(name=f"bcps{rep}", bufs=1, space="PSUM") as bcps, \
             tc.tile_pool(name=f"abuf{rep}", bufs=3) as abuf:
            for half in range(2):
                for g in (2 * half, 2 * half + 1):
                    tiles = sched[g]
                    cps = ctxps.tile([DH + 1, 512], F32, name="cps")
                    pairs = [tiles[i:i + 2] for i in range(0, len(tiles), 2)]
                    nt = 0
                    for pair in pairs:
                        w = 512 * len(pair)
                        sps = scoreps.tile([128, 1024], F32, name="sps")
                        for m, (t, mtype, u) in enumerate(pair):
                            nc.tensor.matmul(sps[:, 512 * m:512 * (m + 1)],
                                             kT[:, 128 * t:128 * (t + 1)],
                                             qT[:, 512 * g:512 * (g + 1)],
                                             start=True, stop=True)
                        p_sb = abuf.tile([128, 1024], BF16, name="p_sb")
                        nc.scalar.activation(p_sb[:, 0:w], sps[:, 0:w],
                                             mybir.ActivationFunctionType.Exp,
                                             scale=0.125)
                        _me = getattr(nc, MASK_ENG)
                        for m, (t, mtype, u) in enumerate(pair):
                            pv = p_sb[:, 512 * m:512 * (m + 1)]
                            if mtype == 1:
                                # pv = (anchor > kv_idx) * pv in one op
                                _me.scalar_tensor_tensor(
                                    pv, anchorb[:, 512 * g:512 * (g + 1)],
                                    kviota[:, t:t + 1], pv,
                                    mybir.AluOpType.is_gt, mybir.AluOpType.mult)
                            elif mtype == 2:
                                _me.tensor_tensor(
                                    pv, pv, dmask[:, 512 * u:512 * (u + 1)],
                                    mybir.AluOpType.mult)
                        for m, (t, mtype, u) in enumerate(pair):
                            nc.tensor.matmul(cps[:], vaug[:, 65 * t:65 * (t + 1)],
                                             p_sb[:, 512 * m:512 * (m + 1)],
                                             start=(nt == 0),
                                             stop=(nt == len(tiles) - 1))
                            nt += 1
                    nc.vector.tensor_copy(ctxT[:, 512 * g:512 * (g + 1)], cps[:])
                    nc.vector.reciprocal(recip[:, 512 * g:512 * (g + 1)],
                                         ctxT[DH:DH + 1, 512 * g:512 * (g + 1)])
                # normalize + AllGather for this half
                hs_ = slice(1024 * half, 1024 * (half + 1))
                bps = bcps.tile([DH, Q // 2], F32, name="bps")
                for j in range(2):
                    jj = 1024 * half + 512 * j
                    nc.tensor.matmul(bps[:, 512 * j:512 * (j + 1)], ones64[:],
                                     recip[:, jj:jj + 512], start=True, stop=True)
                nc.vector.tensor_tensor(gin[:, hs_], ctxT[0:DH, hs_], bps[:],
                                        mybir.AluOpType.mult)
                nc.sync.dma_start(gb_in[half][:], gin[:, hs_])
                if collective:
                    nc.gpsimd.collective_compute(
                        "AllGather", mybir.AluOpType.bypass,
                        replica_groups=[list(range(NC))],
                        ins=[gb_in[half].opt()], outs=[gb_out[half].opt()])
                else:  # timing-model variant: fake the gather with local DMAs
                    for _c in range(NC):
                        nc.sync.dma_start(gb_out[half][DH * _c:DH * (_c + 1), :],
                                          gb_in[half][:])
                for f in range(NF):
                    nc.sync.dma_start(ctxfT[f][:, hs_],
                                      gb_out[half][128 * f:128 * (f + 1), :])

        if PHASE == "attn":
            nc.vector.memset(se_sb[:], 1.0)
            nc.vector.memset(mx_sb[:], 1.0)
            nc.vector.memset(tl_sb[:], 1.0)
            nc.sync.dma_start(o_tl[:], tl_sb[:])
            nc.sync.dma_start(o_se[:], se_sb[:])
            nc.sync.dma_start(o_mx[:], mx_sb[:])
            return

        # ---- per-half: Wo + tlogit, then lm head
        for half in range(2):
            with tc.tile_pool(name=f"wops{rep}_{half}", bufs=2, space="PSUM") as wops, \
                 tc.tile_pool(name=f"tlps{rep}_{half}", bufs=2, space="PSUM") as tlps, \
                 tc.tile_pool(name=f"stbuf{rep}_{half}", bufs=2) as stbuf:
                for fo in range(NF):
                    for g in (2 * half, 2 * half + 1):
                        ps = wops.tile([128, 512], F32, name="wps")
                        for ki in range(NF):
                            nc.tensor.matmul(
                                ps[:],
                                wo_sb[:, D * ki + 128 * fo:D * ki + 128 * (fo + 1)],
                                ctxfT[ki][:, 512 * g:512 * (g + 1)],
                                start=(ki == 0), stop=(ki == NF - 1))
                        nc.scalar.copy(outT[fo][:, 512 * g:512 * (g + 1)], ps[:])
                for j in (2 * half, 2 * half + 1):
                    ps = tlps.tile([1, 512], F32, name="tlp")
                    for f in range(NF):
                        mmc = stbuf.tile([128, 512], F32, name="mmc", tag="mmc")
                        nc.vector.tensor_tensor(
                            mmc[:], outT[f][:, 512 * j:512 * (j + 1)],
                            wt_sb[:, Q * f + 512 * j:Q * f + 512 * (j + 1)],
                            mybir.AluOpType.mult)
                        nc.tensor.matmul(ps[:], onescol_f[:], mmc[:],
                                         start=(f == 0), stop=(f == NF - 1))
                    nc.scalar.copy(tl_sb[:, 512 * j:512 * (j + 1)], ps[:])

            # lm head: [128, 1024] psum tiles, two 500-wide chunks at elem
            # offsets 0/512 (bank-aligned); exp/max via strided views
            with tc.tile_pool(name=f"zps{rep}_{half}", bufs=3, space="PSUM") as zps, \
                 tc.tile_pool(name=f"zbuf{rep}_{half}", bufs=3) as zbuf, \
                 tc.tile_pool(name=f"stbuf2{rep}_{half}", bufs=2) as stbuf2:
                NH = NZC // 2
                for i in range(8 * half, 8 * (half + 1)):
                    se4 = stbuf2.tile([128, NH], F32, name="se4", tag="se4")
                    mx4 = stbuf2.tile([128, NH], F32, name="mx4", tag="mx4")
                    for c4 in range(NH):
                        ps = zps.tile([128, 1024], F32, name="zp")
                        for h in range(2):
                            for f in range(NF):
                                nc.tensor.matmul(
                                    ps[:, 512 * h:512 * h + ZC],
                                    outT[f][:, 128 * i:128 * (i + 1)],
                                    wlm[f][:, ZC * (2 * c4 + h):ZC * (2 * c4 + h + 1)],
                                    start=(f == 0), stop=(f == NF - 1))
                        psv = ps.rearrange("p (c w) -> p c w", w=512)[:, :, 0:ZC]
                        ze = zbuf.tile([128, 2 * ZC], BF16, name="ze")
                        zev = ze.rearrange("p (c w) -> p c w", w=ZC)
                        nc.scalar.activation(zev, psv,
                                             mybir.ActivationFunctionType.Exp,
                                             accum_out=se4[:, c4:c4 + 1])
                        nc.vector.tensor_reduce(mx4[:, c4:c4 + 1], psv,
                                                mybir.AxisListType.XY,
                                                mybir.AluOpType.max)
                    nc.vector.tensor_reduce(se_sb[:, i:i + 1], se4[:],
                                            mybir.AxisListType.X,
                                            mybir.AluOpType.add)
                    nc.vector.tensor_reduce(mx_sb[:, i:i + 1], mx4[:],
                                            mybir.AxisListType.X,
                                            mybir.AluOpType.max)
        nc.sync.dma_start(o_tl[:], tl_sb[:])
        nc.sync.dma_start(o_se[:], se_sb[:])
        nc.sync.dma_start(o_mx[:], mx_sb[:])


def _lay4(a):
    """[512, X] -> [128, 4*X] with [p, f*X+j] = a[128*f+p, j], as bf16."""
    x = a.shape[1]
    return np.ascontiguousarray(
        a.reshape(NF, 128, x).transpose(1, 0, 2).reshape(128, NF * x)
    ).astype(BFNP)


def kernel(**inputs):
    ids = np.asarray(inputs["input_ids"])[0].astype(np.int64)        # [S]
    hs = np.asarray(inputs["hidden_states"])[0].astype(np.float32)   # [S, D]
    lmask = np.asarray(inputs["loss_mask"])[0].astype(np.float32)    # [S]
    anc = np.asarray(inputs["anchor_positions"])[0].astype(np.int64)  # [N]
    keep = np.asarray(inputs["block_keep_mask"])[0].astype(bool)     # [N]
    emb = np.asarray(inputs["embed_table"]).astype(np.float32)       # [V, D]
    Wq = np.asarray(inputs["Wq"]).astype(np.float32)
    Wk = np.asarray(inputs["Wk"]).astype(np.float32)
    Wv = np.asarray(inputs["Wv"]).astype(np.float32)
    Wo = np.asarray(inputs["Wo"]).astype(np.float32)
    Wlm = np.asarray(inputs["W_lm"]).astype(np.float32)

    # ---- host layout prep (index gathers, transposes, casts, slicing) ----
    safe_anchor = np.clip(anc, 0, S - 1)
    start_tokens = np.where(keep, ids[safe_anchor], MASK_TOKEN_ID)
    E_start = emb[start_tokens]                     # [N, D]
    e_mask = emb[MASK_TOKEN_ID]                     # [D]

    offs = np.arange(BS)
    label_idx = anc[:, None] + offs[None, :]        # [N, BS]
    valid = (label_idx < S)
    safe_idx = np.clip(label_idx, 0, S - 1)
    targets = ids[safe_idx].reshape(-1)             # [Q]
    w = (keep[:, None] * valid * (offs > 0)[None, :]
         * lmask[safe_idx]).astype(np.float32).reshape(-1)

    hT = np.ascontiguousarray(hs.T).astype(BFNP)                    # [D, S]
    estt = _lay4(np.ascontiguousarray(E_start.T))                   # [128, 4*128]
    emask4 = np.ascontiguousarray(e_mask.reshape(NF, 128).T).astype(np.float32)
    anchorb = np.ascontiguousarray(
        np.broadcast_to(np.repeat(anc, BS).astype(np.float32)[None, :], (128, Q)))
    kviota = (np.arange(128, dtype=np.float32)[:, None]
              + 128.0 * np.arange(32, dtype=np.float32)[None, :])
    p_idx = np.arange(128)[:, None]
    f_idx = np.arange(512)[None, :]
    dmask = np.concatenate(
        [((f_idx // BS) == (8 * u + p_idx // BS)).astype(np.float32)
         for u in range(4)], axis=1).astype(BFNP)                   # [128, 4*512]
    wt = _lay4(Wlm[:, targets])                                     # [128, 4*Q]
    wo4 = _lay4(Wo)

    key = (anc.tobytes(), 1)
    if key not in _cache:
        _cache[key] = _build_program(_build_schedule(anc))
    nc = _cache[key]

    in_maps = []
    for c in range(NC):
        in_maps.append({
            "i_ht": hT, "i_estt": estt, "i_emask": emask4,
            "i_anchorb": anchorb, "i_kviota": kviota, "i_dmask": dmask,
            "i_wq": _lay4(Wq[:, DH * c:DH * (c + 1)]),
            "i_wk": _lay4(Wk[:, DH * c:DH * (c + 1)]),
            "i_wv": _lay4(Wv[:, DH * c:DH * (c + 1)]),
            "i_wo": wo4,
            "i_wlm": _lay4(Wlm[:, VS * c:VS * (c + 1)]),
            "i_wt": wt,
        })

    global _last_in_maps
    _last_in_maps = in_maps
    res = run_bass_kernel_spmd(nc, in_maps, core_ids=list(range(NC)))

    # ---- host combine ----
    se = np.zeros((128, 16), np.float64)
    mx = np.full((128, 16), -np.inf, np.float32)
    for c in range(NC):
        se += res.results[c]["o_se"].astype(np.float64)
        mx = np.maximum(mx, res.results[c]["o_mx"])
    se_q = se.T.reshape(-1)           # q = 128*i + p
    mx_q = mx.T.reshape(-1)
    tl_q = res.results[0]["o_tl"][0]

    lse = np.log(se_q)
    loss_per = np.where(w > 0, lse - tl_q, 0.0)
    loss = (loss_per * w).sum() / (w.sum() + 1e-6)
    correct = (tl_q >= mx_q - 3e-4) & (w > 0.5)
    acc = correct.sum() / (w.sum() + 1e-6)
    return np.float32(loss), np.float32(acc)



# revision 16
# speedup vs baseline: 1.5844x; 1.5844x over previous
"""DFlash draft-model kernel for 8x Trainium2 NeuronCores.

Sharding: head-parallel attention (core c owns head c) + vocab-parallel LM head
(core c owns vocab shard c), joined by an AllGather of the normalized per-head
context (fp8). Block-sparse attention: kv tiles above each q-group's max
anchor are skipped; within scheduled tiles, dead query columns (sorted
anchors => dead prefix) are trimmed from the score matmul / exp / PV.

fp8 (e4m3) DoubleRow matmuls carry projections, PV, Wo and the LM head;
scales are folded into activation scales and the softmax reciprocal
(ones-column = 2.0 => denom = 2*sum(p), recip * ctx*32 = ctx*16).

Per-core device outputs: row sum(exp) (f32) + row max(exp) (bf16-accurate) of
its logit shard, and the target-column logits; host combines into
(loss, accuracy).
"""
import sys
sys.path.insert(0, '/opt/trn_rl_repo')
import numpy as np
import ml_dtypes

import concourse.mybir as mybir
import concourse.tile as tile
from concourse import bacc
from concourse.bass_utils import run_bass_kernel_spmd
from concourse.bass_interp import get_hw_module

F32 = mybir.dt.float32
BF16 = mybir.dt.bfloat16
FP16 = mybir.dt.float16
F8 = mybir.dt.float8e4
BFNP = ml_dtypes.bfloat16
F8NP = ml_dtypes.float8_e4m3
DR = mybir.MatmulPerfMode.DoubleRow

B, S, N, BS, D, H, V = 1, 2048, 128, 16, 512, 8, 32000
MASK_TOKEN_ID = 3
NC = 8
DH = D // H            # 64
Q = N * BS             # 2048
VS = V // NC           # 4000 vocab per core
QG = 4                 # q groups of 512
ZC = 500               # logits psum chunk (bank-aligned at 512 offsets)

# fp8 scale plan (see module docstring)
SX = 16.0   # x (hidden/noise-emb) scale
SW = 32.0   # Wq/Wk/Wv scale
SK = 8.0    # k/q storage scale -> scores psum = s*64
SV = 32.0   # v scale in vaug
SG = 16.0   # gathered ctx scale (via ones-col 2.0)
SWO = 32.0  # Wo scale -> Wo psum = out*512
SO = 8.0    # outT storage scale
SL = 32.0   # W_lm scale -> lm psum = z*256

_cache = {}
_last_in_maps = None


def _build_schedule(anc):
    """Per q-group: ctx kv tile pairs [(t, masked, bmask_idx)], pair dead-col a0."""
    sched = []
    nmask = 0
    for g in range(QG):
        blk = anc[32 * g:32 * g + 32]          # anchors of this group's blocks
        amin, amax = int(blk.min()), int(blk.max())
        ctx = []
        for t in range((amax + 127) // 128):
            masked = (128 * t + 128) > amin
            # dead prefix: q columns whose anchor <= 128*t (block granular)
            a = int((blk <= 128 * t).sum()) * BS
            mi = -1
            if masked:
                mi = nmask
                nmask += 1
            ctx.append((t, 1 if masked else 0, a, mi))
        pairs = [ctx[i:i + 2] for i in range(0, len(ctx), 2)]
        sched.append(pairs)
    return sched, nmask


def _build_program(sched, nmask, reps=1, collective=True):
    nc = bacc.Bacc("TRN2", target_bir_lowering=False, debug=False, num_devices=NC)

    din = {}
    for name, shape, dt in [
        ("i_bmask", [128, nmask * 512], F8),  # boundary masks (anchor > kv)
        ("i_dmask", [128, 512], F8),       # draft block-diag pattern x4
        ("i_wq", [128, 256], F8),          # [p, fp*128 + j*64 + d]
        ("i_wk", [128, 256], F8),
        ("i_wv", [128, 256], F8),
        ("i_xt0", [128, 2 * (S + Q)], F8),  # X^T*SX rows 0..255   [p, j*4096+x]
        ("i_xt1", [128, 2 * (S + Q)], F8),  # rows 256..511
        ("i_wo0", [128, 2 * D], F8),       # [p, j*512 + o]
        ("i_wo1", [128, 2 * D], F8),
        ("i_wt", [128, 4 * Q], BF16),      # Wlm[:,targets]/SO  [p, f*2048+q]
        ("i_wlm0", [128, 2 * VS], F8),     # [p, j*4000 + v]
        ("i_wlm1", [128, 2 * VS], F8),
    ]:
        din[name] = nc.dram_tensor(name, shape, dt, kind="ExternalInput").ap()
    o_se = nc.dram_tensor("o_se", [128, 16], F32, kind="ExternalOutput").ap()
    o_mx = nc.dram_tensor("o_mx", [128, 16], F32, kind="ExternalOutput").ap()
    o_tl = nc.dram_tensor("o_tl", [1, Q], F32, kind="ExternalOutput").ap()

    with tile.TileContext(nc) as tc:
        for _rep in range(reps):
            _emit(nc, tc, din, o_se, o_mx, o_tl, sched, nmask, collective, _rep)

    nc.compile()
    nc.m = get_hw_module(nc.m)
    return nc


def _emit(nc, tc, din, o_se, o_mx, o_tl, sched, nmask, collective, rep):
    with tc.tile_pool(name=f"persist{rep}", bufs=1) as pp, \
         tc.tile_pool(name=f"dram{rep}", bufs=1, space="DRAM") as dp:
        # ---- loads: attention-side first, lm-head weights last
        bmask = pp.tile([128, max(1, nmask) * 512], F8, name="bmask")
        nc.sync.dma_start(bmask[:], din["i_bmask"][:])
        dmask = pp.tile([128, 512], F8, name="dmask")
        nc.sync.dma_start(dmask[:], din["i_dmask"][:])
        wq_sb = pp.tile([128, 256], F8, name="wq_sb")
        nc.sync.dma_start(wq_sb[:], din["i_wq"][:])
        wk_sb = pp.tile([128, 256], F8, name="wk_sb")
        nc.sync.dma_start(wk_sb[:], din["i_wk"][:])
        wv_sb = pp.tile([128, 256], F8, name="wv_sb")
        nc.sync.dma_start(wv_sb[:], din["i_wv"][:])
        xt = []
        for f in range(2):
            t = pp.tile([128, 2 * (S + Q)], F8, name=f"xt{f}")
            nc.sync.dma_start(t[:], din[f"i_xt{f}"][:])
            xt.append(t)
        wo = []
        for f in range(2):
            t = pp.tile([128, 2 * D], F8, name=f"wo{f}")
            nc.sync.dma_start(t[:], din[f"i_wo{f}"][:])
            wo.append(t)
        wt_sb = pp.tile([128, 4 * Q], BF16, name="wt_sb")
        nc.sync.dma_start(wt_sb[:], din["i_wt"][:])
        wlm = []
        for f in range(2):
            t = pp.tile([128, 2 * VS], F8, name=f"wlm{f}")
            nc.sync.dma_start(t[:], din[f"i_wlm{f}"][:])
            wlm.append(t)

        xtv = [t.rearrange("p (j x) -> p j x", j=2) for t in xt]
        wqv = wq_sb.rearrange("p (f j d) -> p f j d", f=2, j=2)
        wkv = wk_sb.rearrange("p (f j d) -> p f j d", f=2, j=2)
        wvv = wv_sb.rearrange("p (f j d) -> p f j d", f=2, j=2)
        wov = [t.rearrange("p (j o) -> p j o", j=2) for t in wo]
        wlmv = [t.rearrange("p (j v) -> p j v", j=2) for t in wlm]
        wtv = wt_sb.rearrange("p (f q) -> p f q", f=4)

        ones64 = pp.tile([1, DH], BF16, name="ones64")
        nc.vector.memset(ones64[:], 1.0)
        onescol = pp.tile([128, 1], BF16, name="onescol")
        nc.vector.memset(onescol[:], 1.0)

        kT = pp.tile([DH, S + Q], BF16, name="kT")
        qT = pp.tile([DH, Q], BF16, name="qT")
        vaug = pp.tile([128, 32 * 68], F8, name="vaug")
        vav = vaug.rearrange("p (t c) -> p t c", c=68)
        nc.vector.memset(vav[:, :, DH:DH + 1], 2.0)   # denom ones-col (=2)
        nc.vector.memset(vav[:, :, DH + 1:68], 0.0)   # fp8-align padding
        gin = pp.tile([DH, Q], F8, name="gin")
        recip = pp.tile([1, Q], BF16, name="recip")
        bcs = pp.tile([DH, 512], BF16, name="bcs")
        ctxf = [pp.tile([128, 2 * Q], F8, name=f"ctxf{f}") for f in range(2)]
        ctxfv = [t.rearrange("p (j q) -> p j q", j=2) for t in ctxf]
        outT = [pp.tile([128, 2 * Q], F8, name=f"outT{f}") for f in range(2)]
        outTv = [t.rearrange("p (j q) -> p j q", j=2) for t in outT]
        se_sb = pp.tile([128, 16], F32, name="se_sb")
        mx_sb = pp.tile([128, 16], F32, name="mx_sb")
        tl_sb = pp.tile([1, Q], F32, name="tl_sb")
        gb_in = [dp.tile([DH, Q // 2], F8, name=f"gb_in{h}") for h in range(2)]
        gb_out = [dp.tile([NC * DH, Q // 2], F8, name=f"gb_out{h}",
                          addr_space="Shared" if collective else "Local")
                  for h in range(2)]

        # ---- projections (DoubleRow fp8)
        with tc.tile_pool(name=f"projps{rep}", bufs=2, space="PSUM") as projps:
            for n in range((S + Q) // 512):
                ps = projps.tile([DH, 512], F32, name="kps", tag="proj")
                for f in range(2):
                    nc.tensor.matmul(ps[:], wkv[:, f], xtv[f][:, :, 512 * n:512 * (n + 1)],
                                     start=(f == 0), stop=(f == 1), perf_mode=DR)
                nc.vector.tensor_scalar(kT[:, 512 * n:512 * (n + 1)], ps[:],
                                        SK / (SX * SW), None, mybir.AluOpType.mult)
            for n in range(Q // 512):
                ps = projps.tile([DH, 512], F32, name="qps", tag="proj")
                for f in range(2):
                    nc.tensor.matmul(ps[:], wqv[:, f],
                                     xtv[f][:, :, S + 512 * n:S + 512 * (n + 1)],
                                     start=(f == 0), stop=(f == 1), perf_mode=DR)
                nc.vector.tensor_scalar(qT[:, 512 * n:512 * (n + 1)], ps[:],
                                        SK / (SX * SW), None, mybir.AluOpType.mult)
            for T in range(32):
                ps = projps.tile([128, DH], F32, name="vps", tag="proj")
                for f in range(2):
                    nc.tensor.matmul(ps[:], xtv[f][:, :, 128 * T:128 * (T + 1)],
                                     wvv[:, f], start=(f == 0), stop=(f == 1),
                                     perf_mode=DR)
                nc.scalar.mul(vav[:, T, 0:DH], ps[:], SV / (SX * SW))

        # ---- attention: two-half pipeline with per-half AllGather
        with tc.tile_pool(name=f"scoreps{rep}", bufs=2, space="PSUM") as scoreps, \
             tc.tile_pool(name=f"ctxps{rep}", bufs=3, space="PSUM") as ctxps, \
             tc.tile_pool(name=f"bcps{rep}", bufs=1, space="PSUM") as bcps, \
             tc.tile_pool(name=f"abuf{rep}", bufs=3) as abuf:
            for half in range(2):
                for g in (2 * half, 2 * half + 1):
                    pairs = sched[g]
                    qs = 512 * g
                    cps = ctxps.tile([68, 512], F32, name="cps")
                    # draft tiles first (start=True per 128-col region)
                    dps = scoreps.tile([128, 512], F32, name="dsps", tag="sc")
                    for u in range(4):
                        t = 16 + 4 * g + u
                        nc.tensor.matmul(dps[:, 128 * u:128 * (u + 1)],
                                         kT[:, 128 * t:128 * (t + 1)],
                                         qT[:, qs + 128 * u:qs + 128 * (u + 1)],
                                         start=True, stop=True)
                    dp_sb = abuf.tile([128, 1024], F8, name="p_sb", tag="p")
                    nc.scalar.activation(dp_sb[:, 0:512], dps[:],
                                         mybir.ActivationFunctionType.Exp,
                                         scale=0.125 / (SK * SK))
                    nc.gpsimd.tensor_tensor(dp_sb[:, 0:512], dp_sb[:, 0:512],
                                            dmask[:], mybir.AluOpType.mult)
                    for u in range(4):
                        t = 16 + 4 * g + u
                        nc.tensor.matmul(cps[:, 128 * u:128 * (u + 1)],
                                         vav[:, t], dp_sb[:, 128 * u:128 * (u + 1)],
                                         start=True, stop=True,
                                         skip_group_check=True)
                    # ctx tiles in pairs, trimmed to live cols
                    for pi, pair in enumerate(pairs):
                        a0 = pair[0][2]
                        sps = scoreps.tile([128, 1024], F32, name="sps", tag="sc")
                        for m, (t, mtype, a, mi) in enumerate(pair):
                            nc.tensor.matmul(sps[:, 512 * m + a:512 * m + 512],
                                             kT[:, 128 * t:128 * (t + 1)],
                                             qT[:, qs + a:qs + 512],
                                             start=True, stop=True)
                        p_sb = abuf.tile([128, 1024], F8, name="p_sb", tag="p")
                        spv = sps.rearrange("p (m w) -> p m w", m=2)
                        ppv = p_sb.rearrange("p (m w) -> p m w", m=2)
                        nc.scalar.activation(ppv[:, 0:len(pair), a0:512],
                                             spv[:, 0:len(pair), a0:512],
                                             mybir.ActivationFunctionType.Exp,
                                             scale=0.125 / (SK * SK))
                        for m, (t, mtype, a, mi) in enumerate(pair):
                            if mtype:
                                pv = p_sb[:, 512 * m + a0:512 * m + 512]
                                nc.gpsimd.tensor_tensor(
                                    pv, pv, bmask[:, 512 * mi + a0:512 * mi + 512],
                                    mybir.AluOpType.mult)
                        for m, (t, mtype, a, mi) in enumerate(pair):
                            nc.tensor.matmul(cps[:, a0:512],
                                             vav[:, t],
                                             p_sb[:, 512 * m + a0:512 * m + 512],
                                             start=False,
                                             stop=(pi == len(pairs) - 1
                                                   and m == len(pair) - 1),
                                             skip_group_check=True)
                    # normalize: recip of (2*sum p) => ctx*SG via *32 values
                    with nc.allow_low_precision(reason="bf16 recip of denom"):
                        nc.vector.reciprocal(recip[:, qs:qs + 512],
                                             cps[DH:DH + 1, :])
                    bps = bcps.tile([DH, 512], F32, name="bps")
                    nc.tensor.matmul(bps[:], ones64[:], recip[:, qs:qs + 512],
                                     start=True, stop=True)
                    nc.scalar.copy(bcs[:], bps[:])
                    nc.vector.tensor_tensor(gin[:, qs:qs + 512], cps[0:DH, :],
                                            bcs[:], mybir.AluOpType.mult)
                # AllGather for this half
                hs_ = slice(1024 * half, 1024 * (half + 1))
                nc.sync.dma_start(gb_in[half][:], gin[:, hs_])
                if collective:
                    nc.gpsimd.collective_compute(
                        "AllGather", mybir.AluOpType.bypass,
                        replica_groups=[list(range(NC))],
                        ins=[gb_in[half].opt()], outs=[gb_out[half].opt()])
                else:  # timing-model variant: fake the gather with local DMAs
                    for _c in range(NC):
                        nc.sync.dma_start(gb_out[half][DH * _c:DH * (_c + 1), :],
                                          gb_in[half][:])
                for f in range(2):
                    for j in range(2):
                        nc.sync.dma_start(
                            ctxfv[f][:, j, 1024 * half:1024 * (half + 1)],
                            gb_out[half][256 * f + 128 * j:256 * f + 128 * (j + 1), :])

        # ---- Wo (DoubleRow fp8) + target logits, all 4 groups
        with tc.tile_pool(name=f"wops{rep}", bufs=2, space="PSUM") as wops, \
             tc.tile_pool(name=f"tlps{rep}", bufs=2, space="PSUM") as tlps, \
             tc.tile_pool(name=f"stbuf{rep}", bufs=2) as stbuf:
            for g in range(QG):
                for fo in range(4):
                    ps = wops.tile([128, 512], F32, name="wps")
                    for f in range(2):
                        nc.tensor.matmul(
                            ps[:], wov[f][:, :, 128 * fo:128 * (fo + 1)],
                            ctxfv[f][:, :, 512 * g:512 * (g + 1)],
                            start=(f == 0), stop=(f == 1), perf_mode=DR)
                    nc.scalar.mul(outTv[fo // 2][:, fo % 2, 512 * g:512 * (g + 1)],
                                  ps[:], SO / (SG * SWO))
            for g in range(QG):
                ps = tlps.tile([1, 512], F32, name="tlp")
                for fo in range(4):
                    mmc = stbuf.tile([128, 512], BF16, name="mmc", tag="mmc")
                    nc.gpsimd.tensor_tensor(
                        mmc[:], outTv[fo // 2][:, fo % 2, 512 * g:512 * (g + 1)],
                        wtv[:, fo, 512 * g:512 * (g + 1)], mybir.AluOpType.mult)
                    nc.tensor.matmul(ps[:], onescol[:], mmc[:],
                                     start=(fo == 0), stop=(fo == 3))
                nc.scalar.copy(tl_sb[:, 512 * g:512 * (g + 1)], ps[:])

        # ---- LM head: 16 q-chunks x 4000 vocab, fp8 DoubleRow, exp+accum,
        #      bf16 tree-max (DVE 2x; Pool helps on odd chunks)
        with tc.tile_pool(name=f"zps{rep}", bufs=2, space="PSUM") as zps, \
             tc.tile_pool(name=f"zbuf{rep}", bufs=4) as zbuf, \
             tc.tile_pool(name=f"mxbuf{rep}", bufs=2) as mxbuf, \
             tc.tile_pool(name=f"stbuf2{rep}", bufs=2) as stbuf2:
            for i in range(16):
                se2 = stbuf2.tile([128, 2], F32, name="se2", tag="se2")
                zes = []
                for c2 in range(2):
                    ps = zps.tile([128, 2048], F32, name="zp")
                    for f in range(2):
                        for h in range(4):
                            nc.tensor.matmul(
                                ps[:, 512 * h:512 * h + ZC],
                                outTv[f][:, :, 128 * i:128 * (i + 1)],
                                wlmv[f][:, :, 2000 * c2 + ZC * h:2000 * c2 + ZC * (h + 1)],
                                start=(f == 0), stop=(f == 1),
                                perf_mode=DR, skip_group_check=True)
                    ze = zbuf.tile([128, 2048], BF16, name="ze")
                    psv = ps.rearrange("p (c w) -> p c w", w=512)[:, :, 0:ZC]
                    zev = ze.rearrange("p (c w) -> p c w", w=512)[:, :, 0:ZC]
                    nc.scalar.activation(zev, psv,
                                         mybir.ActivationFunctionType.Exp,
                                         scale=1.0 / (SO * SL),
                                         accum_out=se2[:, c2:c2 + 1])
                    zes.append(ze)
                nc.vector.tensor_tensor(se_sb[:, i:i + 1], se2[:, 0:1], se2[:, 1:2],
                                        mybir.AluOpType.add)
                # tree max over 2x[128, 4x500(str512)] bf16
                zm = [mxbuf.tile([128, 1024], BF16, name="zm", tag="zm")
                      for _ in range(2)]
                for c2 in range(2):
                    zv = zes[c2].rearrange("p (c w) -> p c w", w=512)[:, :, 0:ZC]
                    zmv = zm[c2].rearrange("p (c w) -> p c w", w=512)[:, :, 0:ZC]
                    nc.vector.tensor_tensor(zmv, zv[:, 0:2], zv[:, 2:4],
                                            mybir.AluOpType.max)
                z3 = zm[0].rearrange("p (c w) -> p c w", w=512)[:, :, 0:ZC]
                z4 = zm[1].rearrange("p (c w) -> p c w", w=512)[:, :, 0:ZC]
                nc.vector.tensor_tensor(z3, z3, z4, mybir.AluOpType.max)
                nc.vector.tensor_tensor(zm[0][:, 0:ZC], zm[0][:, 0:ZC],
                                        zm[0][:, 512:512 + ZC],
                                        mybir.AluOpType.max)
                nc.vector.tensor_reduce(mx_sb[:, i:i + 1], zm[0][:, 0:ZC],
                                        mybir.AxisListType.X, mybir.AluOpType.max)
        nc.sync.dma_start(o_tl[:], tl_sb[:])
        nc.sync.dma_start(o_se[:], se_sb[:])
        nc.sync.dma_start(o_mx[:], mx_sb[:])


def _pack2(a, scale):
    """[512, X] f32 -> 2 fp8 arrays [128, 2*X]: arr_fp[p, j*X+x] = a[256fp+128j+p, x]."""
    x = a.shape[1]
    r = (a * scale).reshape(2, 2, 128, x).astype(F8NP)
    return [np.ascontiguousarray(r[fp].transpose(1, 0, 2).reshape(128, 2 * x))
            for fp in range(2)]


def kernel(**inputs):
    ids = np.asarray(inputs["input_ids"])[0].astype(np.int64)        # [S]
    hs = np.asarray(inputs["hidden_states"])[0].astype(np.float32)   # [S, D]
    lmask = np.asarray(inputs["loss_mask"])[0].astype(np.float32)    # [S]
    anc = np.asarray(inputs["anchor_positions"])[0].astype(np.int64)  # [N]
    keep = np.asarray(inputs["block_keep_mask"])[0].astype(bool)     # [N]
    emb = np.asarray(inputs["embed_table"]).astype(np.float32)       # [V, D]
    Wq = np.asarray(inputs["Wq"]).astype(np.float32)
    Wk = np.asarray(inputs["Wk"]).astype(np.float32)
    Wv = np.asarray(inputs["Wv"]).astype(np.float32)
    Wo = np.asarray(inputs["Wo"]).astype(np.float32)
    Wlm = np.asarray(inputs["W_lm"]).astype(np.float32)

    # ---- host layout prep ----
    safe_anchor = np.clip(anc, 0, S - 1)
    start_tokens = np.where(keep, ids[safe_anchor], MASK_TOKEN_ID)
    ne = np.tile(emb[MASK_TOKEN_ID], (Q, 1))
    ne[0::BS] = emb[start_tokens]                   # [Q, D]
    xt_full = np.concatenate([hs, ne], 0).T         # [D, S+Q]

    offs = np.arange(BS)
    label_idx = anc[:, None] + offs[None, :]
    valid = (label_idx < S)
    safe_idx = np.clip(label_idx, 0, S - 1)
    targets = ids[safe_idx].reshape(-1)             # [Q]
    w = (keep[:, None] * valid * (offs > 0)[None, :]
         * lmask[safe_idx]).astype(np.float32).reshape(-1)

    xt8 = _pack2(xt_full, SX)
    p_idx = np.arange(128)[:, None]
    j_idx = np.arange(512)[None, :]
    dmask = ((p_idx // BS) == (j_idx % 128) // BS).astype(np.float32).astype(F8NP)
    sched, nmask = _build_schedule(anc)
    anchor_q = np.repeat(anc, BS)                    # [Q]
    bmask = np.zeros((128, max(1, nmask) * 512), np.float32)
    for g in range(QG):
        for pair in sched[g]:
            for (t, mtype, a, mi) in pair:
                if mtype:
                    kv = 128 * t + np.arange(128)[:, None]
                    av = anchor_q[None, 512 * g:512 * (g + 1)]
                    bmask[:, 512 * mi:512 * (mi + 1)] = (av > kv)
    bmask = bmask.astype(F8NP)
    wt = np.ascontiguousarray(
        (Wlm[:, targets] / SO).reshape(4, 128, Q).transpose(1, 0, 2)
        .reshape(128, 4 * Q)).astype(BFNP)
    wo8 = _pack2(Wo, SWO)
    wlm8 = {}
    for c in range(NC):
        wlm8[c] = _pack2(Wlm[:, VS * c:VS * (c + 1)], SL)

    key = (anc.tobytes(), 3)
    if key not in _cache:
        _cache[key] = _build_program(sched, nmask)
    nc = _cache[key]

    in_maps = []
    for c in range(NC):
        wq8 = _pack2(Wq[:, DH * c:DH * (c + 1)], SW)
        wk8 = _pack2(Wk[:, DH * c:DH * (c + 1)], SW)
        wv8 = _pack2(Wv[:, DH * c:DH * (c + 1)], SW)
        in_maps.append({
            "i_bmask": bmask, "i_dmask": dmask,
            "i_wq": np.concatenate(wq8, 1), "i_wk": np.concatenate(wk8, 1),
            "i_wv": np.concatenate(wv8, 1),
            "i_xt0": xt8[0], "i_xt1": xt8[1],
            "i_wo0": wo8[0], "i_wo1": wo8[1],
            "i_wt": wt,
            "i_wlm0": wlm8[c][0], "i_wlm1": wlm8[c][1],
        })

    global _last_in_maps
    _last_in_maps = in_maps
    res = run_bass_kernel_spmd(nc, in_maps, core_ids=list(range(NC)))

    # ---- host combine ----
    se = np.zeros((128, 16), np.float64)
    mx = np.zeros((128, 16), np.float32)
    for c in range(NC):
        se += res.results[c]["o_se"].astype(np.float64)
        mx = np.maximum(mx, res.results[c]["o_mx"])
    se_q = se.T.reshape(-1)           # q = 128*i + p
    mx_q = mx.T.reshape(-1)           # max of exp(z)
    tl_q = res.results[0]["o_tl"][0]

    lse = np.log(se_q)
    loss_per = np.where(w > 0, lse - tl_q, 0.0)
    loss = (loss_per * w).sum() / (w.sum() + 1e-6)
    correct = (tl_q >= np.log(np.maximum(mx_q, 1e-30)) - 3e-4) & (w > 0.5)
    acc = correct.sum() / (w.sum() + 1e-6)
    return np.float32(loss), np.float32(acc)


# revision 19
# speedup vs baseline: 2.0523x; 1.2953x over previous
"""DFlash draft-model kernel for 8x Trainium2 NeuronCores.

Sharding: head-parallel attention (core c owns head c) + vocab-parallel LM head
(core c owns vocab shard c), joined by an AllGather of the normalized per-head
context (fp8). Block-sparse attention: kv tiles above each q-group's max
anchor are skipped; within scheduled tiles, dead query columns (sorted
anchors => dead prefix) are trimmed from the score matmul / exp / PV.

fp8 (e4m3) DoubleRow matmuls carry projections, PV, Wo and the LM head;
scales are folded into activation scales and the softmax reciprocal
(ones-column = 2.0 => denom = 2*sum(p), recip * ctx*32 = ctx*16).

Per-core device outputs: row sum(exp) (f32) + row max(exp) (bf16-accurate) of
its logit shard, and the target-column logits; host combines into
(loss, accuracy).
"""
import sys
sys.path.insert(0, '/opt/trn_rl_repo')
import numpy as np
import ml_dtypes

import concourse.mybir as mybir
import concourse.tile as tile
from concourse import bacc
from concourse.bass_utils import run_bass_kernel_spmd
from concourse.bass_interp import get_hw_module

F32 = mybir.dt.float32
BF16 = mybir.dt.bfloat16
FP16 = mybir.dt.float16
F8 = mybir.dt.float8e4
BFNP = ml_dtypes.bfloat16
F8NP = ml_dtypes.float8_e4m3
DR = mybir.MatmulPerfMode.DoubleRow

B, S, N, BS, D, H, V = 1, 2048, 128, 16, 512, 8, 32000
MASK_TOKEN_ID = 3
NC = 8
DH = D // H            # 64
Q = N * BS             # 2048
VS = V // NC           # 4000 vocab per core
QG = 4                 # q groups of 512
ZC = 500               # logits psum chunk (bank-aligned at 512 offsets)

# fp8 scale plan (see module docstring)
SX = 16.0   # x (hidden/noise-emb) scale
SW = 32.0   # Wq/Wk/Wv scale
SK = 8.0    # k/q storage scale -> scores psum = s*64
SV = 32.0   # v scale in vaug
SG = 16.0   # gathered ctx scale (via ones-col 2.0)
SWO = 32.0  # Wo scale -> Wo psum = out*512
SO = 8.0    # outT storage scale
SL = 32.0   # W_lm scale -> lm psum = z*256

_cache = {}
_last_in_maps = None


def _build_schedule(anc):
    """Per q-group: ctx kv tile pairs [(t, masked, bmask_idx)], pair dead-col a0."""
    sched = []
    nmask = 0
    for g in range(QG):
        blk = anc[32 * g:32 * g + 32]          # anchors of this group's blocks
        amin, amax = int(blk.min()), int(blk.max())
        ctx = []
        for t in range((amax + 127) // 128):
            masked = (128 * t + 128) > amin
            # dead prefix: q columns whose anchor <= 128*t (block granular)
            a = int((blk <= 128 * t).sum()) * BS
            mi = -1
            if masked:
                mi = nmask
                nmask += 1
            ctx.append((t, 1 if masked else 0, a, mi))
        pairs = [ctx[i:i + 2] for i in range(0, len(ctx), 2)]
        sched.append(pairs)
    return sched, nmask


def _build_program(sched, nmask, reps=1, collective=True):
    nc = bacc.Bacc("TRN2", target_bir_lowering=False, debug=False, num_devices=NC)

    din = {}
    for name, shape, dt in [
        ("i_bmask", [128, nmask * 512], F8),  # boundary masks (anchor > kv)
        ("i_dmask", [128, 512], F8),       # draft block-diag pattern x4
        ("i_wq", [128, 256], F8),          # [p, fp*128 + j*64 + d]
        ("i_wk", [128, 256], F8),
        ("i_wv", [128, 256], F8),
        ("i_xt0", [128, 2 * (S + Q)], F8),  # X^T*SX rows 0..255   [p, j*4096+x]
        ("i_xt1", [128, 2 * (S + Q)], F8),  # rows 256..511
        ("i_wo0", [128, 2 * D], F8),       # [p, j*512 + o]
        ("i_wo1", [128, 2 * D], F8),
        ("i_wt", [128, 4 * Q], BF16),      # Wlm[:,targets]/SO  [p, f*2048+q]
        ("i_wlm0", [128, 2 * VS], F8),     # [p, j*4000 + v]
        ("i_wlm1", [128, 2 * VS], F8),
    ]:
        din[name] = nc.dram_tensor(name, shape, dt, kind="ExternalInput").ap()
    o_se = nc.dram_tensor("o_se", [128, 16], F32, kind="ExternalOutput").ap()
    o_mx = nc.dram_tensor("o_mx", [128, 16], F32, kind="ExternalOutput").ap()
    o_tl = nc.dram_tensor("o_tl", [1, Q], F32, kind="ExternalOutput").ap()

    with tile.TileContext(nc) as tc:
        for _rep in range(reps):
            _emit(nc, tc, din, o_se, o_mx, o_tl, sched, nmask, collective, _rep)

    nc.compile()
    nc.m = get_hw_module(nc.m)
    return nc


def _emit(nc, tc, din, o_se, o_mx, o_tl, sched, nmask, collective, rep):
    with tc.tile_pool(name=f"persist{rep}", bufs=1) as pp, \
         tc.tile_pool(name=f"dram{rep}", bufs=1, space="DRAM") as dp:
        # ---- loads: projection inputs first, masks next, lm-head weights last
        wq_sb = pp.tile([128, 256], F8, name="wq_sb")
        nc.sync.dma_start(wq_sb[:], din["i_wq"][:])
        wk_sb = pp.tile([128, 256], F8, name="wk_sb")
        nc.sync.dma_start(wk_sb[:], din["i_wk"][:])
        wv_sb = pp.tile([128, 256], F8, name="wv_sb")
        nc.sync.dma_start(wv_sb[:], din["i_wv"][:])
        xt = []
        for f in range(2):
            t = pp.tile([128, 2 * (S + Q)], F8, name=f"xt{f}")
            nc.sync.dma_start(t[:], din[f"i_xt{f}"][:])
            xt.append(t)
        bmask = pp.tile([128, max(1, nmask) * 512], F8, name="bmask")
        nc.sync.dma_start(bmask[:], din["i_bmask"][:])
        dmask = pp.tile([128, 512], F8, name="dmask")
        nc.sync.dma_start(dmask[:], din["i_dmask"][:])
        wo = []
        for f in range(2):
            t = pp.tile([128, 2 * D], F8, name=f"wo{f}")
            nc.sync.dma_start(t[:], din[f"i_wo{f}"][:])
            wo.append(t)
        wt_sb = pp.tile([128, 4 * Q], BF16, name="wt_sb")
        nc.sync.dma_start(wt_sb[:], din["i_wt"][:])
        wlm = []
        for f in range(2):
            t = pp.tile([128, 2 * VS], F8, name=f"wlm{f}")
            nc.sync.dma_start(t[:], din[f"i_wlm{f}"][:])
            wlm.append(t)

        xtv = [t.rearrange("p (j x) -> p j x", j=2) for t in xt]
        wqv = wq_sb.rearrange("p (f j d) -> p f j d", f=2, j=2)
        wkv = wk_sb.rearrange("p (f j d) -> p f j d", f=2, j=2)
        wvv = wv_sb.rearrange("p (f j d) -> p f j d", f=2, j=2)
        wov = [t.rearrange("p (j o) -> p j o", j=2) for t in wo]
        wlmv = [t.rearrange("p (j v) -> p j v", j=2) for t in wlm]
        wtv = wt_sb.rearrange("p (f q) -> p f q", f=4)

        ones64 = pp.tile([1, DH], BF16, name="ones64")
        nc.vector.memset(ones64[:], 1.0)
        onescol = pp.tile([128, 1], BF16, name="onescol")
        nc.vector.memset(onescol[:], 1.0)

        kT = pp.tile([DH, S + Q], BF16, name="kT")
        qT = pp.tile([DH, Q], BF16, name="qT")
        vaug = pp.tile([128, 32 * 68], F8, name="vaug")
        vav = vaug.rearrange("p (t c) -> p t c", c=68)
        nc.vector.memset(vav[:, :, DH:DH + 1], 2.0)   # denom ones-col (=2)
        nc.vector.memset(vav[:, :, DH + 1:68], 0.0)   # fp8-align padding
        gin = pp.tile([DH, Q], F8, name="gin")
        recip = pp.tile([1, Q], BF16, name="recip")
        bcs = pp.tile([DH, 512], BF16, name="bcs")
        ctxf = [pp.tile([128, 2 * Q], F8, name=f"ctxf{f}") for f in range(2)]
        ctxfv = [t.rearrange("p (j q) -> p j q", j=2) for t in ctxf]
        outT = [pp.tile([128, 2 * Q], F8, name=f"outT{f}") for f in range(2)]
        outTv = [t.rearrange("p (j q) -> p j q", j=2) for t in outT]
        se_sb = pp.tile([128, 16], F32, name="se_sb")
        mx_sb = pp.tile([128, 16], F32, name="mx_sb")
        tl_sb = pp.tile([1, Q], F32, name="tl_sb")
        gb_in = [dp.tile([DH, Q // 2], F8, name=f"gb_in{h}") for h in range(2)]
        gb_out = [dp.tile([NC * DH, Q // 2], F8, name=f"gb_out{h}",
                          addr_space="Shared" if collective else "Local")
                  for h in range(2)]

        # ---- projections (DoubleRow fp8)
        with tc.tile_pool(name=f"projps{rep}", bufs=2, space="PSUM") as projps:
            for n in range((S + Q) // 512):
                ps = projps.tile([DH, 512], F32, name="kps", tag="proj")
                for f in range(2):
                    nc.tensor.matmul(ps[:], wkv[:, f], xtv[f][:, :, 512 * n:512 * (n + 1)],
                                     start=(f == 0), stop=(f == 1), perf_mode=DR)
                nc.vector.tensor_scalar(kT[:, 512 * n:512 * (n + 1)], ps[:],
                                        SK / (SX * SW), None, mybir.AluOpType.mult)
            for n in range(Q // 512):
                ps = projps.tile([DH, 512], F32, name="qps", tag="proj")
                for f in range(2):
                    nc.tensor.matmul(ps[:], wqv[:, f],
                                     xtv[f][:, :, S + 512 * n:S + 512 * (n + 1)],
                                     start=(f == 0), stop=(f == 1), perf_mode=DR)
                nc.vector.tensor_scalar(qT[:, 512 * n:512 * (n + 1)], ps[:],
                                        SK / (SX * SW), None, mybir.AluOpType.mult)
            for T in range(32):
                ps = projps.tile([128, DH], F32, name="vps", tag="proj")
                for f in range(2):
                    nc.tensor.matmul(ps[:], xtv[f][:, :, 128 * T:128 * (T + 1)],
                                     wvv[:, f], start=(f == 0), stop=(f == 1),
                                     perf_mode=DR)
                nc.scalar.mul(vav[:, T, 0:DH], ps[:], SV / (SX * SW))

        # ---- attention: two-half pipeline with per-half AllGather
        with tc.tile_pool(name=f"scoreps{rep}", bufs=2, space="PSUM") as scoreps, \
             tc.tile_pool(name=f"ctxps{rep}", bufs=3, space="PSUM") as ctxps, \
             tc.tile_pool(name=f"bcps{rep}", bufs=1, space="PSUM") as bcps, \
             tc.tile_pool(name=f"abuf{rep}", bufs=4) as abuf:
            for half in range(2):
                for g in (2 * half, 2 * half + 1):
                    pairs = sched[g]
                    qs = 512 * g
                    cps = ctxps.tile([68, 512], F32, name="cps")
                    # draft tiles first (start=True per 128-col region)
                    dps = scoreps.tile([128, 512], F32, name="dsps", tag="sc")
                    for u in range(4):
                        t = 16 + 4 * g + u
                        nc.tensor.matmul(dps[:, 128 * u:128 * (u + 1)],
                                         kT[:, 128 * t:128 * (t + 1)],
                                         qT[:, qs + 128 * u:qs + 128 * (u + 1)],
                                         start=True, stop=True)
                    dp_sb = abuf.tile([128, 1024], F8, name="p_sb", tag="p")
                    nc.scalar.activation(dp_sb[:, 0:512], dps[:],
                                         mybir.ActivationFunctionType.Exp,
                                         scale=0.125 / (SK * SK))
                    nc.gpsimd.tensor_tensor(dp_sb[:, 0:512], dp_sb[:, 0:512],
                                            dmask[:], mybir.AluOpType.mult)
                    for u in range(4):
                        t = 16 + 4 * g + u
                        nc.tensor.matmul(cps[:, 128 * u:128 * (u + 1)],
                                         vav[:, t], dp_sb[:, 128 * u:128 * (u + 1)],
                                         start=True, stop=True,
                                         skip_group_check=True)
                    # ctx tiles in pairs, trimmed to live cols
                    for pi, pair in enumerate(pairs):
                        a0 = pair[0][2]
                        sps = scoreps.tile([128, 1024], F32, name="sps", tag="sc")
                        for m, (t, mtype, a, mi) in enumerate(pair):
                            nc.tensor.matmul(sps[:, 512 * m + a:512 * m + 512],
                                             kT[:, 128 * t:128 * (t + 1)],
                                             qT[:, qs + a:qs + 512],
                                             start=True, stop=True)
                        p_sb = abuf.tile([128, 1024], F8, name="p_sb", tag="p")
                        spv = sps.rearrange("p (m w) -> p m w", m=2)
                        ppv = p_sb.rearrange("p (m w) -> p m w", m=2)
                        nc.scalar.activation(ppv[:, 0:len(pair), a0:512],
                                             spv[:, 0:len(pair), a0:512],
                                             mybir.ActivationFunctionType.Exp,
                                             scale=0.125 / (SK * SK))
                        for m, (t, mtype, a, mi) in enumerate(pair):
                            if mtype:
                                pv = p_sb[:, 512 * m + a0:512 * m + 512]
                                nc.gpsimd.tensor_tensor(
                                    pv, pv, bmask[:, 512 * mi + a0:512 * mi + 512],
                                    mybir.AluOpType.mult)
                        for m, (t, mtype, a, mi) in enumerate(pair):
                            nc.tensor.matmul(cps[:, a0:512],
                                             vav[:, t],
                                             p_sb[:, 512 * m + a0:512 * m + 512],
                                             start=False,
                                             stop=(pi == len(pairs) - 1
                                                   and m == len(pair) - 1),
                                             skip_group_check=True)
                    # normalize: recip of (2*sum p) => ctx*SG via *32 values
                    with nc.allow_low_precision(reason="bf16 recip of denom"):
                        nc.vector.reciprocal(recip[:, qs:qs + 512],
                                             cps[DH:DH + 1, :])
                    bps = bcps.tile([DH, 512], F32, name="bps")
                    nc.tensor.matmul(bps[:], ones64[:], recip[:, qs:qs + 512],
                                     start=True, stop=True)
                    nc.vector.tensor_copy(bcs[:], bps[:])
                    nc.vector.tensor_tensor(gin[:, qs:qs + 512], cps[0:DH, :],
                                            bcs[:], mybir.AluOpType.mult)
                # AllGather for this half
                hs_ = slice(1024 * half, 1024 * (half + 1))
                nc.sync.dma_start(gb_in[half][:], gin[:, hs_])
                if collective:
                    nc.gpsimd.collective_compute(
                        "AllGather", mybir.AluOpType.bypass,
                        replica_groups=[list(range(NC))],
                        ins=[gb_in[half].opt()], outs=[gb_out[half].opt()])
                else:  # timing-model variant: fake the gather with local DMAs
                    for _c in range(NC):
                        nc.sync.dma_start(gb_out[half][DH * _c:DH * (_c + 1), :],
                                          gb_in[half][:])
                for f in range(2):
                    for j in range(2):
                        nc.sync.dma_start(
                            ctxfv[f][:, j, 1024 * half:1024 * (half + 1)],
                            gb_out[half][256 * f + 128 * j:256 * f + 128 * (j + 1), :])

        # ---- Wo (DoubleRow fp8) + target logits, all 4 groups
        with tc.tile_pool(name=f"wops{rep}", bufs=2, space="PSUM") as wops, \
             tc.tile_pool(name=f"tlps{rep}", bufs=2, space="PSUM") as tlps, \
             tc.tile_pool(name=f"stbuf{rep}", bufs=2) as stbuf:
            for g in range(QG):
                for fo in range(4):
                    ps = wops.tile([128, 512], F32, name="wps")
                    for f in range(2):
                        nc.tensor.matmul(
                            ps[:], wov[f][:, :, 128 * fo:128 * (fo + 1)],
                            ctxfv[f][:, :, 512 * g:512 * (g + 1)],
                            start=(f == 0), stop=(f == 1), perf_mode=DR)
                    nc.vector.tensor_scalar(
                        outTv[fo // 2][:, fo % 2, 512 * g:512 * (g + 1)],
                        ps[:], SO / (SG * SWO), None, mybir.AluOpType.mult)
            for g in range(QG):
                ps = tlps.tile([1, 512], F32, name="tlp")
                for fo in range(4):
                    mmc = stbuf.tile([128, 512], BF16, name="mmc", tag="mmc")
                    nc.gpsimd.tensor_tensor(
                        mmc[:], outTv[fo // 2][:, fo % 2, 512 * g:512 * (g + 1)],
                        wtv[:, fo, 512 * g:512 * (g + 1)], mybir.AluOpType.mult)
                    nc.tensor.matmul(ps[:], onescol[:], mmc[:],
                                     start=(fo == 0), stop=(fo == 3))
                nc.vector.tensor_copy(tl_sb[:, 512 * g:512 * (g + 1)], ps[:])

        # ---- LM head: 16 q-chunks x 4000 vocab, fp8 DoubleRow, exp+accum,
        #      bf16 tree-max (DVE 2x; Pool helps on odd chunks)
        with tc.tile_pool(name=f"zps{rep}", bufs=2, space="PSUM") as zps, \
             tc.tile_pool(name=f"zbuf{rep}", bufs=4) as zbuf, \
             tc.tile_pool(name=f"mxbuf{rep}", bufs=4) as mxbuf, \
             tc.tile_pool(name=f"stbuf2{rep}", bufs=2) as stbuf2:
            for i in range(16):
                se2 = stbuf2.tile([128, 2], F32, name="se2", tag="se2")
                zes = []
                for c2 in range(2):
                    ps = zps.tile([128, 2048], F32, name="zp")
                    for f in range(2):
                        for h in range(4):
                            nc.tensor.matmul(
                                ps[:, 512 * h:512 * h + ZC],
                                outTv[f][:, :, 128 * i:128 * (i + 1)],
                                wlmv[f][:, :, 2000 * c2 + ZC * h:2000 * c2 + ZC * (h + 1)],
                                start=(f == 0), stop=(f == 1),
                                perf_mode=DR, skip_group_check=True)
                    ze = zbuf.tile([128, 2048], BF16, name="ze")
                    psv = ps.rearrange("p (c w) -> p c w", w=512)[:, :, 0:ZC]
                    zev = ze.rearrange("p (c w) -> p c w", w=512)[:, :, 0:ZC]
                    nc.scalar.activation(zev, psv,
                                         mybir.ActivationFunctionType.Exp,
                                         scale=1.0 / (SO * SL),
                                         accum_out=se2[:, c2:c2 + 1])
                    zes.append(ze)
                nc.vector.tensor_tensor(se_sb[:, i:i + 1], se2[:, 0:1], se2[:, 1:2],
                                        mybir.AluOpType.add)
                # tree max over 2x[128, 4x500(str512)] bf16
                zm = [mxbuf.tile([128, 1024], BF16, name="zm", tag="zm")
                      for _ in range(2)]
                for c2 in range(2):
                    zv = zes[c2].rearrange("p (c w) -> p c w", w=512)[:, :, 0:ZC]
                    zmv = zm[c2].rearrange("p (c w) -> p c w", w=512)[:, :, 0:ZC]
                    nc.vector.tensor_tensor(zmv, zv[:, 0:2], zv[:, 2:4],
                                            mybir.AluOpType.max)
                z3 = zm[0].rearrange("p (c w) -> p c w", w=512)[:, :, 0:ZC]
                z4 = zm[1].rearrange("p (c w) -> p c w", w=512)[:, :, 0:ZC]
                nc.vector.tensor_tensor(z3, z3, z4, mybir.AluOpType.max)
                nc.vector.tensor_tensor(zm[0][:, 0:ZC], zm[0][:, 0:ZC],
                                        zm[0][:, 512:512 + ZC],
                                        mybir.AluOpType.max)
                nc.vector.tensor_reduce(mx_sb[:, i:i + 1], zm[0][:, 0:ZC],
                                        mybir.AxisListType.X, mybir.AluOpType.max)
        nc.sync.dma_start(o_tl[:], tl_sb[:])
        nc.sync.dma_start(o_se[:], se_sb[:])
        nc.sync.dma_start(o_mx[:], mx_sb[:])


def _pack2(a, scale):
    """[512, X] f32 -> 2 fp8 arrays [128, 2*X]: arr_fp[p, j*X+x] = a[256fp+128j+p, x]."""
    x = a.shape[1]
    r = (a * scale).reshape(2, 2, 128, x).astype(F8NP)
    return [np.ascontiguousarray(r[fp].transpose(1, 0, 2).reshape(128, 2 * x))
            for fp in range(2)]


def kernel(**inputs):
    ids = np.asarray(inputs["input_ids"])[0].astype(np.int64)        # [S]
    hs = np.asarray(inputs["hidden_states"])[0].astype(np.float32)   # [S, D]
    lmask = np.asarray(inputs["loss_mask"])[0].astype(np.float32)    # [S]
    anc = np.asarray(inputs["anchor_positions"])[0].astype(np.int64)  # [N]
    keep = np.asarray(inputs["block_keep_mask"])[0].astype(bool)     # [N]
    emb = np.asarray(inputs["embed_table"]).astype(np.float32)       # [V, D]
    Wq = np.asarray(inputs["Wq"]).astype(np.float32)
    Wk = np.asarray(inputs["Wk"]).astype(np.float32)
    Wv = np.asarray(inputs["Wv"]).astype(np.float32)
    Wo = np.asarray(inputs["Wo"]).astype(np.float32)
    Wlm = np.asarray(inputs["W_lm"]).astype(np.float32)

    # ---- host layout prep ----
    safe_anchor = np.clip(anc, 0, S - 1)
    start_tokens = np.where(keep, ids[safe_anchor], MASK_TOKEN_ID)
    ne = np.tile(emb[MASK_TOKEN_ID], (Q, 1))
    ne[0::BS] = emb[start_tokens]                   # [Q, D]
    xt_full = np.concatenate([hs, ne], 0).T         # [D, S+Q]

    offs = np.arange(BS)
    label_idx = anc[:, None] + offs[None, :]
    valid = (label_idx < S)
    safe_idx = np.clip(label_idx, 0, S - 1)
    targets = ids[safe_idx].reshape(-1)             # [Q]
    w = (keep[:, None] * valid * (offs > 0)[None, :]
         * lmask[safe_idx]).astype(np.float32).reshape(-1)

    xt8 = _pack2(xt_full, SX)
    p_idx = np.arange(128)[:, None]
    j_idx = np.arange(512)[None, :]
    dmask = ((p_idx // BS) == (j_idx % 128) // BS).astype(np.float32).astype(F8NP)
    sched, nmask = _build_schedule(anc)
    anchor_q = np.repeat(anc, BS)                    # [Q]
    bmask = np.zeros((128, max(1, nmask) * 512), np.float32)
    for g in range(QG):
        for pair in sched[g]:
            for (t, mtype, a, mi) in pair:
                if mtype:
                    kv = 128 * t + np.arange(128)[:, None]
                    av = anchor_q[None, 512 * g:512 * (g + 1)]
                    bmask[:, 512 * mi:512 * (mi + 1)] = (av > kv)
    bmask = bmask.astype(F8NP)
    wt = np.ascontiguousarray(
        (Wlm[:, targets] / SO).reshape(4, 128, Q).transpose(1, 0, 2)
        .reshape(128, 4 * Q)).astype(BFNP)
    wo8 = _pack2(Wo, SWO)
    wlm8 = {}
    for c in range(NC):
        wlm8[c] = _pack2(Wlm[:, VS * c:VS * (c + 1)], SL)

    key = (anc.tobytes(), 3)
    if key not in _cache:
        _cache[key] = _build_program(sched, nmask)
    nc = _cache[key]

    in_maps = []
    for c in range(NC):
        wq8 = _pack2(Wq[:, DH * c:DH * (c + 1)], SW)
        wk8 = _pack2(Wk[:, DH * c:DH * (c + 1)], SW)
        wv8 = _pack2(Wv[:, DH * c:DH * (c + 1)], SW)
        in_maps.append({
            "i_bmask": bmask, "i_dmask": dmask,
            "i_wq": np.concatenate(wq8, 1), "i_wk": np.concatenate(wk8, 1),
            "i_wv": np.concatenate(wv8, 1),
            "i_xt0": xt8[0], "i_xt1": xt8[1],
            "i_wo0": wo8[0], "i_wo1": wo8[1],
            "i_wt": wt,
            "i_wlm0": wlm8[c][0], "i_wlm1": wlm8[c][1],
        })

    global _last_in_maps
    _last_in_maps = in_maps
    res = run_bass_kernel_spmd(nc, in_maps, core_ids=list(range(NC)))

    # ---- host combine ----
    se = np.zeros((128, 16), np.float64)
    mx = np.zeros((128, 16), np.float32)
    for c in range(NC):
        se += res.results[c]["o_se"].astype(np.float64)
        mx = np.maximum(mx, res.results[c]["o_mx"])
    se_q = se.T.reshape(-1)           # q = 128*i + p
    mx_q = mx.T.reshape(-1)           # max of exp(z)
    tl_q = res.results[0]["o_tl"][0]

    lse = np.log(se_q)
    loss_per = np.where(w > 0, lse - tl_q, 0.0)
    loss = (loss_per * w).sum() / (w.sum() + 1e-6)
    correct = (tl_q >= np.log(np.maximum(mx_q, 1e-30)) - 3e-4) & (w > 0.5)
    acc = correct.sum() / (w.sum() + 1e-6)
    return np.float32(loss), np.float32(acc)
